# revision 1
# baseline (speedup 1.0000x reference)
"""Cross-attention kernel for Trainium2, 8-core tensor-parallel over heads.

Problem (fixed shapes, fp32):
    patch_embed [2, 2048, 1024], pixel_embed [2, 2048, 1024]
    Wq/Wk/Wv [1024, 1024], Wo [1024, 1024], bo [1024]
    16 heads x 64 dim_head, softmax cross-attention, out [2, 2048, 1024].

Sharding: core c handles batch b = c // 4 and head-group g = c % 4
(4 heads = 256 inner cols). Each core computes a partial output
(its heads' contribution to out @ Wo); host sums the 4 partials per
batch and adds the bias.

Per-core device program (matmuls in fp32r: full PE rate, ~3e-4 rel err):
    pixelT/patchT: [seq, d] -> [d, seq] via hi/lo-u16 DMA transposes
        (fp32 xbar transpose is unsupported; two 2-byte transposes +
        a DVE interleave-merge reconstruct the fp32 transpose)
    kT = Wk_g^T @ pixelT                     [256, m]
    v  = pixelT^T @ Wv_g  (+ ones column)    [m, 4, 65]
    qT = Wq_g^T @ patchT                     [256, n]
    per head: sT = kT_h^T @ qT_h  (K=64, row-packed head pairs)
              eT = exp(scale * sT)           (ACT, 2 key tiles per op)
              oT[65] += v_aug^T @ eT         (row 64 accumulates Z)
              oT_n = oT[0:64] * (1/Z)        (gpsimd partition_broadcast)
    y_partial = oT_n^T @ Wo_g                [n, 1024], DMA PSUM->DRAM
"""

import numpy as np

HEADS = 16
DH = 64
B = 2
N = 2048          # query seq len
M = 2048          # key seq len
D = 1024
N_CORES = 8
HPC = 4           # heads per core
C = HPC * DH      # 256 inner cols per core
SCALE = DH ** -0.5
P = 128
FREE = 512        # fp32 matmul moving free dim (one PSUM bank)
KT_D = D // P     # 8 contraction tiles for projections
ST = M // P       # 16 seq tiles
QC = N // FREE    # 4 query chunks
JT = M // P       # 16 key tiles

USE_DMA_T = True  # u16 DMA-transpose path; False = PE-transpose fallback

import os
STAGE = os.environ.get("K_STAGE", "full")  # proj | attn | nonorm | full

_cache = {}


def _build_nc():
    import concourse.bacc as bacc
    import concourse.mybir as mybir
    import concourse.tile as tile
    from concourse.masks import make_identity

    F32 = mybir.dt.float32
    F32R = mybir.dt.float32r
    U16 = mybir.dt.uint16
    EXP = mybir.ActivationFunctionType.Exp

    nc = bacc.Bacc("TRN2", target_bir_lowering=False, debug=False,
                   num_devices=N_CORES)

    if USE_DMA_T:
        # hi/lo u16 planes of patch/pixel, split host-side (fp32 bit halves)
        pel = nc.dram_tensor("pel", [N, D], mybir.dt.uint16,
                             kind="ExternalInput")
        peh = nc.dram_tensor("peh", [N, D], mybir.dt.uint16,
                             kind="ExternalInput")
        xel = nc.dram_tensor("xel", [M, D], mybir.dt.uint16,
                             kind="ExternalInput")
        xeh = nc.dram_tensor("xeh", [M, D], mybir.dt.uint16,
                             kind="ExternalInput")
    else:
        pe = nc.dram_tensor("pe", [N, D], F32, kind="ExternalInput")
        xe = nc.dram_tensor("xe", [M, D], F32, kind="ExternalInput")
    wq = nc.dram_tensor("wq", [D, C], F32, kind="ExternalInput")
    wk = nc.dram_tensor("wk", [D, C], F32, kind="ExternalInput")
    wv = nc.dram_tensor("wv", [D, C], F32, kind="ExternalInput")
    wo = nc.dram_tensor("wo", [C, D], F32, kind="ExternalInput")
    yp = nc.dram_tensor("yp", [N, D], F32, kind="ExternalOutput")

    if not USE_DMA_T:
        pe_t = pe.ap().rearrange("(st p) d -> st p d", p=P)  # [16,128,1024]
        xe_t = xe.ap().rearrange("(st p) d -> st p d", p=P)
    wq_t = wq.ap().rearrange("(ko ki) c -> ki ko c", ki=P)   # [128,8,256]
    wk_t = wk.ap().rearrange("(ko ki) c -> ki ko c", ki=P)
    wv_t = wv.ap().rearrange("(ko ki) c -> ki ko c", ki=P)
    wo_t = wo.ap().rearrange("(ko ki) n -> ki ko n", ki=P)   # [128,2,1024]
    yp_t = yp.ap().rearrange("(qt p) d -> qt p d", p=P)

    with tile.TileContext(nc) as tc:
        with (
            tc.tile_pool(name="const", bufs=1) as const,
            tc.tile_pool(name="wpool", bufs=2) as wpool,
            tc.tile_pool(name="wstage", bufs=1) as wstage,
            tc.tile_pool(name="bigT", bufs=1) as bigT,
            tc.tile_pool(name="qk", bufs=1) as qk,
        ):
            def load_round(dram_ap, shape, name):
                stage = wstage.tile(shape, F32, tag="wstage", name=f"{name}_s")
                nc.sync.dma_start(out=stage[:], in_=dram_ap)
                r = wpool.tile(shape, F32R, tag="w", name=f"{name}_r")
                nc.vector.tensor_copy(r[:], stage[:])
                return r

            wk_r = load_round(wk_t, [P, KT_D, C], "wk")
            wv_r = load_round(wv_t, [P, KT_D, C], "wv")

            v_r = const.tile([P, JT, HPC, DH + 1], F32R, name="v_r")
            qT_r = qk.tile([P, 2, N], F32R, name="qT_r")
            kT_r = qk.tile([P, 2, M], F32R, name="kT_r")
            oT_r = qk.tile([P, 2, N], F32R, name="oT_r")

            if not USE_DMA_T:
                ident = const.tile([P, P], F32)
                make_identity(nc, ident)

            # ---- transpose: dram [seq, d] -> sbuf f32r [d(part), seq] ----
            def transpose_in_dma(src_lo, src_hi, n_seq, name, tpools):
                u16p, stp = tpools
                dstT = bigT.tile([P, KT_D, n_seq], F32R, tag="bigT",
                                 name=name)
                for kt in range(KT_D):
                    lo = u16p.tile([P, n_seq], U16, tag="u16")
                    hi = u16p.tile([P, n_seq], U16, tag="u16")
                    nc.sync.dma_start(
                        out=lo[:], in_=src_lo.ap()[:, kt * P:(kt + 1) * P],
                        transpose=True)
                    nc.sync.dma_start(
                        out=hi[:], in_=src_hi.ap()[:, kt * P:(kt + 1) * P],
                        transpose=True)
                    stage = stp.tile([P, n_seq], F32, tag="tstage")
                    st3 = stage[:].bitcast(U16).rearrange(
                        "p (s two) -> p s two", two=2)
                    nc.vector.tensor_copy(st3[:, :, 0], lo[:])
                    nc.vector.tensor_copy(st3[:, :, 1], hi[:])
                    nc.vector.tensor_copy(dstT[:, kt, :], stage[:])
                return dstT

            def transpose_in_pe(src_tiled, n_st, name, tpools):
                natp, ptp = tpools
                dstT = bigT.tile([P, KT_D, n_st * P], F32R, tag="bigT",
                                 name=name)
                for st4 in range(n_st // 4):
                    nats = []
                    for s in range(4):
                        t = natp.tile([P, D], F32, tag="nat")
                        nc.sync.dma_start(out=t[:],
                                          in_=src_tiled[st4 * 4 + s])
                        nats.append(t)
                    for kt in range(KT_D):
                        pt = ptp.tile([P, 4 * P], F32, tag="pt")
                        for s in range(4):
                            nc.tensor.transpose(
                                pt[:, s * P:(s + 1) * P],
                                nats[s][:, kt * P:(kt + 1) * P], ident[:])
                        nc.vector.tensor_copy(
                            dstT[:, kt, st4 * 4 * P:(st4 + 1) * 4 * P],
                            pt[:])
                return dstT

            # ---- projection: out[mt] = W[:, mt].T @ xT, rounded -----------
            def project_T(w_r, xT, out_r, n_seq, ppool):
                for mt in range(2):
                    for q2 in range(n_seq // (2 * FREE)):
                        pp = ppool.tile([P, 2 * FREE], F32, tag="pp")
                        for half in range(2):
                            qc = q2 * 2 + half
                            for kt in range(KT_D):
                                nc.tensor.matmul(
                                    pp[:, half * FREE:(half + 1) * FREE],
                                    w_r[:, kt, mt * P:(mt + 1) * P],
                                    xT[:, kt, qc * FREE:(qc + 1) * FREE],
                                    start=(kt == 0), stop=(kt == KT_D - 1))
                        nc.vector.tensor_copy(
                            out_r[:, mt, q2 * 2 * FREE:(q2 + 1) * 2 * FREE],
                            pp[:])

            # ================= transpose + projection phase ===============
            with (
                tc.tile_pool(name="u16p", bufs=4) as u16p,
                tc.tile_pool(name="stp", bufs=2) as stp,
                tc.tile_pool(name="ppsum", bufs=2, space="PSUM") as ppsum,
            ):
                if USE_DMA_T:
                    pxT = transpose_in_dma(xel, xeh, M, "pxT", (u16p, stp))
                else:
                    pxT = transpose_in_pe(xe_t, ST, "pxT", (u16p, ppsum))
                project_T(wk_r, pxT, kT_r, M, ppsum)
                # V: [m(part), 4h, 64] + ones col
                for st4 in range(ST // 4):
                    pv = ppsum.tile([P, 4 * C], F32, tag="pp")
                    for s in range(4):
                        st = st4 * 4 + s
                        for kt in range(KT_D):
                            nc.tensor.matmul(
                                pv[:, s * C:(s + 1) * C],
                                pxT[:, kt, st * P:(st + 1) * P],
                                wv_r[:, kt, :],
                                start=(kt == 0), stop=(kt == KT_D - 1))
                    nc.vector.tensor_copy(
                        v_r[:, st4 * 4:(st4 + 1) * 4, :, 0:DH],
                        pv[:].rearrange("p (s h e) -> p s h e", s=4, h=HPC))
                ones_f = const.tile([P, JT * HPC], F32, name="ones_f")
                nc.vector.memset(ones_f[:], 1.0)
                nc.vector.tensor_copy(
                    v_r[:, :, :, DH],
                    ones_f[:].rearrange("p (a b) -> p a b", a=JT))

                wq_r = load_round(wq_t, [P, KT_D, C], "wq")
                if USE_DMA_T:
                    paT = transpose_in_dma(pel, peh, N, "paT", (u16p, stp))
                else:
                    paT = transpose_in_pe(pe_t, N // P, "paT", (u16p, ppsum))
                project_T(wq_r, paT, qT_r, N, ppsum)

            # ======================= attention phase ======================
            if STAGE == "proj":
                # time transposes+projections only: dump qT as output
                with tc.tile_pool(name="dmp", bufs=2) as dmp:
                    for qt in range(N // P):
                        d = dmp.tile([P, D], F32, tag="d")
                        nc.vector.tensor_copy(
                            d[:, 0:FREE],
                            qT_r[:, 0, (qt % 4) * FREE:(qt % 4 + 1) * FREE])
                        nc.sync.dma_start(out=yp_t[qt], in_=d[:])
            do_attn = STAGE != "proj"
            with (
                tc.tile_pool(name="eT", bufs=2) as epool,
                tc.tile_pool(name="rzp", bufs=2) as rzp,
                tc.tile_pool(name="rzbp", bufs=2) as rzbp,
                tc.tile_pool(name="spsum", bufs=2, space="PSUM") as spsum,
                tc.tile_pool(name="pacc", bufs=2, space="PSUM") as pacc,
            ):
                for pair in range(2 if do_attn else 0):
                    for qc in range(QC):
                        po = [pacc.tile([DH + 1, FREE], F32, tag="po",
                                        name=f"po{hh}") for hh in range(2)]
                        for jt2 in range(JT // 2):
                            for hh in range(2):
                                h = pair * 2 + hh
                                pst = spsum.tile([P, 2 * FREE], F32,
                                                 tag="ps", name="pst")
                                for k in range(2):
                                    jt = jt2 * 2 + k
                                    nc.tensor.matmul(
                                        pst[:, k * FREE:(k + 1) * FREE],
                                        kT_r[hh * DH:(hh + 1) * DH, pair,
                                             jt * P:(jt + 1) * P],
                                        qT_r[hh * DH:(hh + 1) * DH, pair,
                                             qc * FREE:(qc + 1) * FREE],
                                        start=True, stop=True)
                                eT = epool.tile([P, 2 * FREE], F32R,
                                                tag="eT")
                                if STAGE == "noexp":
                                    nc.vector.tensor_copy(eT[:], pst[:])
                                else:
                                    nc.scalar.activation(eT[:], pst[:], EXP,
                                                         scale=SCALE)
                                for k in range(2):
                                    jt = jt2 * 2 + k
                                    nc.tensor.matmul(
                                        po[hh][:], v_r[:, jt, h, :],
                                        eT[:, k * FREE:(k + 1) * FREE],
                                        start=(jt == 0), stop=(jt == JT - 1))
                        for hh in range(2):
                            if STAGE == "nonorm":
                                nc.vector.tensor_copy(
                                    oT_r[hh * DH:(hh + 1) * DH, pair,
                                         qc * FREE:(qc + 1) * FREE],
                                    po[hh][0:DH, :])
                                continue
                            rz = rzp.tile([1, FREE], F32, tag="rz")
                            nc.vector.reciprocal(rz[:],
                                                 po[hh][DH:DH + 1, :])
                            rzb = rzbp.tile([DH, FREE], F32, tag="rzb")
                            nc.gpsimd.partition_broadcast(rzb[:], rz[:])
                            nc.vector.tensor_mul(
                                oT_r[hh * DH:(hh + 1) * DH, pair,
                                     qc * FREE:(qc + 1) * FREE],
                                po[hh][0:DH, :], rzb[:])

            # ==================== output projection =======================
            wo_r = load_round(wo_t, [P, 2, D], "wo")
            with (
                tc.tile_pool(name="yout", bufs=3) as yout,
                tc.tile_pool(name="ypsum", bufs=2, space="PSUM") as ypsum,
            ):
                for qt in range(N // P if do_attn else 0):
                    py = ypsum.tile([P, D], F32, tag="py")
                    for nk in range(D // FREE):
                        for ct in range(2):
                            nc.tensor.matmul(
                                py[:, nk * FREE:(nk + 1) * FREE],
                                oT_r[:, ct, qt * P:(qt + 1) * P],
                                wo_r[:, ct, nk * FREE:(nk + 1) * FREE],
                                start=(ct == 0), stop=(ct == 1))
                    ysb = yout.tile([P, D], F32, tag="y")
                    nc.vector.tensor_copy(ysb[:], py[:])
                    nc.sync.dma_start(out=yp_t[qt], in_=ysb[:])

    nc.compile()
    return nc


def get_nc():
    if "nc" not in _cache:
        _cache["nc"] = _build_nc()
    return _cache["nc"]


def _split_u16(x):
    """fp32 [n, d] -> (lo, hi) u16 planes of the raw bits (little-endian)."""
    v = np.ascontiguousarray(x, dtype=np.float32).view(np.uint16)
    v = v.reshape(x.shape[0], x.shape[1], 2)
    return np.ascontiguousarray(v[:, :, 0]), np.ascontiguousarray(v[:, :, 1])


def make_core_inputs(patch_embed, pixel_embed, Wq, Wk, Wv, Wo, c):
    b, g = divmod(c, HPC)
    sl = slice(g * C, (g + 1) * C)
    m = {
        "wq": np.ascontiguousarray(Wq[:, sl], dtype=np.float32),
        "wk": np.ascontiguousarray(Wk[:, sl], dtype=np.float32),
        "wv": np.ascontiguousarray(Wv[:, sl], dtype=np.float32),
        "wo": np.ascontiguousarray(Wo[sl, :], dtype=np.float32),
    }
    if USE_DMA_T:
        m["pel"], m["peh"] = _split_u16(patch_embed[b])
        m["xel"], m["xeh"] = _split_u16(pixel_embed[b])
    else:
        m["pe"] = np.ascontiguousarray(patch_embed[b], dtype=np.float32)
        m["xe"] = np.ascontiguousarray(pixel_embed[b], dtype=np.float32)
    return m


def kernel(patch_embed, pixel_embed, Wq, Wk, Wv, Wo, bo):
    from concourse.bass_utils import run_bass_kernel_spmd

    nc = get_nc()
    in_maps = [make_core_inputs(patch_embed, pixel_embed, Wq, Wk, Wv, Wo, c)
               for c in range(N_CORES)]
    res = run_bass_kernel_spmd(nc, in_maps, core_ids=list(range(N_CORES)))
    out = np.empty((B, N, D), dtype=np.float32)
    for b in range(B):
        acc = res.results[b * HPC + 0]["yp"].astype(np.float32)
        for g in range(1, HPC):
            acc = acc + res.results[b * HPC + g]["yp"]
        out[b] = acc + np.asarray(bo, dtype=np.float32)[None, :]
    return out

